# revision 5
# baseline (speedup 1.0000x reference)
"""Trainium2 Bass kernel for causal multi-head attention.

Problem: B=2, S=2048, D=1024, H=16 heads (hd=64), fp32 in/out.
  qkv = x @ Wqkv + bqkv ; per-head causal softmax attention ; out = ctx @ Wo + bo

Sharding (8 NeuronCores): tensor-parallel over heads — 2 heads per core.
Each core computes q/k/v projections for its 2 heads (both batches), causal
attention, and its ctx^T slice [128 feat, B*S]. Two 512KB AllGathers (one per
batch, the first overlapped with batch-1 attention) share ctx^T; each core
then computes the output projection for 256 rows of each batch with the full
Wo. Host reassembles the row slices.

Numerics: bf16 matmul operands, fp32 PSUM accumulation. Softmax uses
exp without max-subtraction (scores are ~N(0,1) after the folded 1/sqrt(hd)
scale; |s| < ~8 so fp32 exp/sums are safe). The softmax denominator comes
for free as a ones-column appended to v in the attn@v matmul.
"""

import numpy as np
import ml_dtypes

B, S, D, H, NC = 2, 2048, 1024, 16, 8
HD = D // H            # 64
HPC = H // NC          # 2 heads per core
BS = B * S             # 4096
RPB = S // NC          # 256 output rows per core per batch
KC = D // 128          # 8 contraction chunks
SC = BS // 512         # 8 s-chunks for qkv projection
NQT = S // 512         # 4 q-tiles (512) per batch
NKT = S // 128         # 16 k-tiles (128) per batch

BF16 = ml_dtypes.bfloat16

_CACHE = {}


def _build_program():
    import concourse.bass as bass
    import concourse.mybir as mybir
    from concourse import bacc
    from concourse.tile import TileContext

    dt = mybir.dt
    f32, bf16 = dt.float32, dt.bfloat16
    ALU = mybir.AluOpType
    ACTF = mybir.ActivationFunctionType

    nc = bacc.Bacc("TRN2", target_bir_lowering=False, debug=False, num_devices=NC)

    xT = nc.dram_tensor("xT", [D, BS], bf16, kind="ExternalInput")
    wqk = nc.dram_tensor("wqk", [D, 256], bf16, kind="ExternalInput")
    wv = nc.dram_tensor("wv", [D, 128], bf16, kind="ExternalInput")
    wo = nc.dram_tensor("wo", [D, D], bf16, kind="ExternalInput")
    bqk = nc.dram_tensor("bqk", [128, 2], f32, kind="ExternalInput")
    bv = nc.dram_tensor("bv", [128, 128], bf16, kind="ExternalInput")
    bo = nc.dram_tensor("bo", [128, D], f32, kind="ExternalInput")
    mask = nc.dram_tensor("mask", [128, 896], bf16, kind="ExternalInput")
    out = nc.dram_tensor("out", [2 * RPB, D], f32, kind="ExternalOutput")

    # per-batch collective buffers
    ctx_dram = [nc.dram_tensor(f"ctxb{b}", [128, S], bf16) for b in range(B)]
    ctxag_dram = [
        nc.dram_tensor(f"ctxag{b}", [NC * 128, S], bf16, addr_space="Shared")
        for b in range(B)
    ]

    with TileContext(nc) as tc:
        with (
            tc.tile_pool(name="const", bufs=1) as cpool,
            tc.tile_pool(name="big", bufs=1) as bigpool,
            tc.tile_pool(name="xstream", bufs=2) as xpool,
            tc.tile_pool(name="exp", bufs=2) as epool,
            tc.tile_pool(name="small", bufs=3) as spool,
            tc.tile_pool(name="outp", bufs=2) as opool,
            tc.tile_pool(name="psA", bufs=2, space="PSUM") as psA,   # 2x [128,1536]
            tc.tile_pool(name="psB", bufs=2, space="PSUM") as psB,   # 2x [128,512]
        ):
            # ---- constants / weights to SBUF ----
            wqk_sb = cpool.tile([128, KC, 256], bf16, tag="wqk")
            nc.sync.dma_start(wqk_sb[:], wqk.rearrange("(ko p) m -> p ko m", p=128))
            wv_sb = cpool.tile([128, KC, 128], bf16, tag="wv")
            nc.sync.dma_start(wv_sb[:], wv.rearrange("(ko p) m -> p ko m", p=128))
            wo_sb = cpool.tile([128, KC, D], bf16, tag="wo")
            nc.sync.dma_start(wo_sb[:], wo.rearrange("(ko p) m -> p ko m", p=128))
            bqk_sb = cpool.tile([128, 2], f32, tag="bqk")
            nc.sync.dma_start(bqk_sb[:], bqk[:])
            bv_sb = cpool.tile([128, 128], bf16, tag="bv")
            nc.sync.dma_start(bv_sb[:], bv[:])
            bo_sb = cpool.tile([128, D], f32, tag="bo")
            nc.sync.dma_start(bo_sb[:], bo[:])
            mask_sb = cpool.tile([128, 896], bf16, tag="mask")
            nc.sync.dma_start(mask_sb[:], mask[:])

            # ---- persistent activations ----
            qT_sb = bigpool.tile([128, BS], bf16, tag="qT")   # [2*64 feat, B*S]
            kT_sb = bigpool.tile([128, BS], bf16, tag="kT")
            # v natural layout + ones cols: per 128-row chunk:
            #   [v_h0(0:64) | ones(64) | v_h1(65:129) | ones(129)]
            v_sb = bigpool.tile([128, BS // 128, 130], bf16, tag="v")
            ctxT_sb = bigpool.tile([128, BS], bf16, tag="ctxT")

            nc.vector.memset(v_sb[:, :, 64:65], 1.0)
            nc.vector.memset(v_sb[:, :, 129:130], 1.0)

            # ---- phase 1: qkv projections ----
            xT_r = xT.rearrange("(ko p) s -> p ko s", p=128)
            for sc in range(SC):
                xt = xpool.tile([128, KC, 512], bf16, tag="xt")
                nc.sync.dma_start(xt[:], xT_r[:, :, sc * 512:(sc + 1) * 512])

                ps_q = psA.tile([128, 1536], f32, tag="psA", name="ps_q")[:, :512]
                ps_k = psA.tile([128, 1536], f32, tag="psA", name="ps_k")[:, :512]
                for kk in range(KC):
                    nc.tensor.matmul(ps_q, lhsT=wqk_sb[:, kk, 0:128],
                                     rhs=xt[:, kk, :],
                                     start=(kk == 0), stop=(kk == KC - 1))
                for kk in range(KC):
                    nc.tensor.matmul(ps_k, lhsT=wqk_sb[:, kk, 128:256],
                                     rhs=xt[:, kk, :],
                                     start=(kk == 0), stop=(kk == KC - 1))
                qs = slice(sc * 512, (sc + 1) * 512)
                nc.vector.tensor_scalar_add(qT_sb[:, qs], ps_q, bqk_sb[:, 0:1])
                nc.vector.tensor_scalar_add(kT_sb[:, qs], ps_k, bqk_sb[:, 1:2])

                for s4 in range(4):
                    sidx = sc * 4 + s4
                    ps_v = psB.tile([128, 512], f32, tag="psB", name="ps_v")[:, :128]
                    for kk in range(KC):
                        nc.tensor.matmul(
                            ps_v,
                            lhsT=xt[:, kk, s4 * 128:(s4 + 1) * 128],
                            rhs=wv_sb[:, kk, :],
                            start=(kk == 0), stop=(kk == KC - 1))
                    nc.vector.tensor_tensor(v_sb[:, sidx, 0:64], ps_v[:, 0:64],
                                            bv_sb[:, 0:64], ALU.add)
                    nc.vector.tensor_tensor(v_sb[:, sidx, 65:129], ps_v[:, 64:128],
                                            bv_sb[:, 64:128], ALU.add)

            # ---- phase 2: attention per (batch, local head) ----
            # scores/exp per k-tile, with the ctx accumulation for q-tile j
            # interleaved right after its last k-tile (t = 4j+3) so the PE
            # instruction stream never stalls on exp availability.
            for b in range(B):
                for hl in range(HPC):
                    hp = slice(64 * hl, 64 * hl + 64)   # feature partitions
                    exp_tiles = []
                    for t in range(NKT):
                        jmin = t // 4
                        width = S - 512 * jmin
                        et = epool.tile([128, width], bf16, tag=f"exp{t}")
                        kT_l = kT_sb[hp, b * S + t * 128: b * S + (t + 1) * 128]
                        off = 0
                        while off < width:
                            pw = min(1536, width - off)
                            ps = psA.tile([128, 1536], f32, tag="psA",
                                          name="ps_sc")[:, :pw]
                            for qi in range(pw // 512):
                                qlo = b * S + 512 * jmin + off + qi * 512
                                nc.tensor.matmul(
                                    ps[:, qi * 512:(qi + 1) * 512],
                                    lhsT=kT_l,
                                    rhs=qT_sb[hp, qlo:qlo + 512],
                                    start=True, stop=True)
                            nc.scalar.activation(et[:, off:off + pw], ps, ACTF.Exp)
                            off += pw
                        # causal mask on the diagonal 512-block
                        mo = 384 - 128 * (t % 4)
                        nc.vector.tensor_tensor(et[:, 0:512], et[:, 0:512],
                                                mask_sb[:, mo:mo + 512], ALU.mult)
                        exp_tiles.append(et)

                        if t % 4 != 3:
                            continue
                        # ctx^T for q-tile j = t//4: all its exp tiles exist
                        j = t // 4
                        ps_c = psB.tile([128, 512], f32, tag="psB", name="ps_c")
                        nkt = 4 * (j + 1)
                        for tt in range(nkt):
                            qoff = 512 * (j - tt // 4)
                            nc.tensor.matmul(
                                ps_c[:65, :],
                                lhsT=v_sb[:, b * NKT + tt, 65 * hl: 65 * hl + 65],
                                rhs=exp_tiles[tt][:, qoff:qoff + 512],
                                start=(tt == 0), stop=(tt == nkt - 1))
                        cs = slice(b * S + j * 512, b * S + (j + 1) * 512)
                        # stage out of PSUM fast, then normalize off the PE path
                        nc.vector.tensor_copy(ctxT_sb[hp, cs], ps_c[0:64, :])
                        den = spool.tile([1, 512], f32, tag="den")
                        nc.vector.tensor_copy(den[:], ps_c[64:65, :])
                        recip = spool.tile([1, 512], f32, tag="recip")
                        nc.vector.reciprocal_approx_fast(out=recip[:], in_=den[:])
                        bcast = spool.tile([128, 512], f32, tag="bcast")
                        nc.gpsimd.partition_broadcast(bcast[:], recip[:])
                        nc.vector.tensor_tensor(ctxT_sb[hp, cs], ctxT_sb[hp, cs],
                                                bcast[hp, :], ALU.mult)

                # batch b ctx^T complete on this core -> all-gather it
                nc.sync.dma_start(ctx_dram[b][:], ctxT_sb[:, b * S:(b + 1) * S])
                nc.gpsimd.collective_compute(
                    "AllGather",
                    mybir.AluOpType.bypass,
                    replica_groups=[list(range(NC))],
                    ins=[ctx_dram[b][:]],
                    outs=[ctxag_dram[b][:]],
                )

            # ---- phase 3: output projection ----
            # core c handles rows [256c, 256c+256) of each batch
            part = nc.partition_id()
            for b in range(B):
                ctxag_sb = bigpool.tile([128, NC, RPB], bf16, tag=f"ctxag{b}",
                                        name="ctxag_sb")
                ctxag_r = ctxag_dram[b].rearrange("(k p) s -> p k s", p=128)
                nc.sync.dma_start(
                    ctxag_sb[:], ctxag_r[:, :, bass.ds(part * RPB, RPB)])

                for rc in range(RPB // 128):
                    ot = opool.tile([128, D], f32, tag="ot")
                    for ncol in range(D // 512):
                        ps_o = psB.tile([128, 512], f32, tag="psB", name="ps_o")
                        for k in range(NC):
                            nc.tensor.matmul(
                                ps_o,
                                lhsT=ctxag_sb[:, k, rc * 128:(rc + 1) * 128],
                                rhs=wo_sb[:, k, ncol * 512:(ncol + 1) * 512],
                                start=(k == 0), stop=(k == NC - 1))
                        nc.vector.tensor_tensor(
                            ot[:, ncol * 512:(ncol + 1) * 512], ps_o,
                            bo_sb[:, ncol * 512:(ncol + 1) * 512], ALU.add)
                    nc.sync.dma_start(
                        out[b * RPB + rc * 128: b * RPB + (rc + 1) * 128, :],
                        ot[:])

    nc.compile()
    return nc


def _prep_inputs(x, Wqkv, bqkv, Wo, bo):
    x = np.asarray(x, dtype=np.float32)
    Wqkv = np.asarray(Wqkv, dtype=np.float32)
    bqkv = np.asarray(bqkv, dtype=np.float32)
    Wo = np.asarray(Wo, dtype=np.float32)
    bo = np.asarray(bo, dtype=np.float32)

    xT = np.ascontiguousarray(x.reshape(BS, D).T).astype(BF16)
    wo_b = Wo.astype(BF16)
    bo_t = np.tile(bo.astype(np.float32), (128, 1))

    kp = np.arange(128)[:, None]
    u = np.arange(896)[None, :]
    mask = (u >= 384 + kp).astype(BF16)

    scale = np.float32(1.0 / np.sqrt(HD))

    # Wqkv columns per head h: q = 192h..+64, k = +64, v = +128
    W3 = Wqkv.reshape(D, H, 3, HD)
    b3 = bqkv.reshape(H, 3, HD)

    in_maps = []
    for c in range(NC):
        hs = [HPC * c + i for i in range(HPC)]
        wq = np.concatenate([W3[:, h, 0, :] for h in hs], axis=1) * scale
        wk = np.concatenate([W3[:, h, 1, :] for h in hs], axis=1)
        wv_ = np.concatenate([W3[:, h, 2, :] for h in hs], axis=1)
        bq = np.concatenate([b3[h, 0, :] for h in hs]) * scale
        bk = np.concatenate([b3[h, 1, :] for h in hs])
        bv_ = np.concatenate([b3[h, 2, :] for h in hs])
        in_maps.append({
            "xT": xT,
            "wqk": np.ascontiguousarray(
                np.concatenate([wq, wk], axis=1)).astype(BF16),
            "wv": np.ascontiguousarray(wv_).astype(BF16),
            "wo": wo_b,
            "bqk": np.ascontiguousarray(
                np.stack([bq, bk], axis=1)).astype(np.float32),
            "bv": np.tile(bv_.astype(BF16), (128, 1)),
            "bo": bo_t,
            "mask": mask,
        })
    return in_maps


def run(x, Wqkv, bqkv, Wo, bo, trace=False):
    from concourse.bass_utils import run_bass_kernel_spmd

    if "nc" not in _CACHE:
        _CACHE["nc"] = _build_program()
    nc = _CACHE["nc"]
    in_maps = _prep_inputs(x, Wqkv, bqkv, Wo, bo)
    res = run_bass_kernel_spmd(nc, in_maps, list(range(NC)), trace=trace)
    # core c returns [2*RPB, D]: rows [256c,256c+256) of batch 0 then batch 1
    full = np.empty((B, S, D), dtype=np.float32)
    for c in range(NC):
        r = res.results[c]["out"]
        for b in range(B):
            full[b, RPB * c: RPB * (c + 1), :] = r[b * RPB:(b + 1) * RPB, :]
    return full, res


def kernel(x, Wqkv, bqkv, Wo, bo):
    out, _ = run(x, Wqkv, bqkv, Wo, bo)
    return out


# revision 6
# speedup vs baseline: 1.0869x; 1.0869x over previous
"""Trainium2 Bass kernel for causal multi-head attention.

Problem: B=2, S=2048, D=1024, H=16 heads (hd=64), fp32 in/out.
  qkv = x @ Wqkv + bqkv ; per-head causal softmax attention ; out = ctx @ Wo + bo

Sharding (8 NeuronCores): tensor-parallel over heads — 2 heads per core.
Each core computes q/k/v projections for its 2 heads (both batches), causal
attention, and its ctx^T slice [128 feat, B*S]. Two 512KB AllGathers (one per
batch, the first overlapped with batch-1 attention) share ctx^T; each core
then computes the output projection for 256 rows of each batch with the full
Wo. Host reassembles the row slices.

Numerics: bf16 matmul operands, fp32 PSUM accumulation. Softmax uses
exp without max-subtraction (scores are ~N(0,1) after the folded 1/sqrt(hd)
scale; |s| < ~8 so fp32 exp/sums are safe). The softmax denominator comes
for free as a ones-column appended to v in the attn@v matmul.
"""

import numpy as np
import ml_dtypes

B, S, D, H, NC = 2, 2048, 1024, 16, 8
HD = D // H            # 64
HPC = H // NC          # 2 heads per core
BS = B * S             # 4096
RPB = S // NC          # 256 output rows per core per batch
KC = D // 128          # 8 contraction chunks
SC = BS // 512         # 8 s-chunks for qkv projection
NQT = S // 512         # 4 q-tiles (512) per batch
NKT = S // 128         # 16 k-tiles (128) per batch

BF16 = ml_dtypes.bfloat16

_CACHE = {}


def _build_program():
    import concourse.bass as bass
    import concourse.mybir as mybir
    from concourse import bacc
    from concourse.tile import TileContext

    dt = mybir.dt
    f32, bf16 = dt.float32, dt.bfloat16
    ALU = mybir.AluOpType
    ACTF = mybir.ActivationFunctionType

    nc = bacc.Bacc("TRN2", target_bir_lowering=False, debug=False, num_devices=NC)

    xT = nc.dram_tensor("xT", [D, BS], bf16, kind="ExternalInput")
    wqk = nc.dram_tensor("wqk", [D, 256], bf16, kind="ExternalInput")
    wv = nc.dram_tensor("wv", [D, 128], bf16, kind="ExternalInput")
    wo = nc.dram_tensor("wo", [D, D], bf16, kind="ExternalInput")
    bqk = nc.dram_tensor("bqk", [128, 2], f32, kind="ExternalInput")
    bv = nc.dram_tensor("bv", [128, 128], bf16, kind="ExternalInput")
    bo = nc.dram_tensor("bo", [128, D], f32, kind="ExternalInput")
    mask = nc.dram_tensor("mask", [128, 896], bf16, kind="ExternalInput")
    out = nc.dram_tensor("out", [2 * RPB, D], f32, kind="ExternalOutput")

    # per-batch collective buffers
    ctx_dram = [nc.dram_tensor(f"ctxb{b}", [128, S], bf16) for b in range(B)]
    ctxag_dram = [
        nc.dram_tensor(f"ctxag{b}", [NC * 128, S], bf16, addr_space="Shared")
        for b in range(B)
    ]

    with TileContext(nc) as tc:
        with (
            tc.tile_pool(name="const", bufs=1) as cpool,
            tc.tile_pool(name="big", bufs=1) as bigpool,
            tc.tile_pool(name="xstream", bufs=2) as xpool,
            tc.tile_pool(name="exp", bufs=2) as epool,
            tc.tile_pool(name="small", bufs=3) as spool,
            tc.tile_pool(name="outp", bufs=2) as opool,
            tc.tile_pool(name="psA", bufs=2, space="PSUM") as psA,   # 2x [128,1536]
            tc.tile_pool(name="psB", bufs=2, space="PSUM") as psB,   # 2x [128,512]
        ):
            # ---- constants / weights to SBUF ----
            wqk_sb = cpool.tile([128, KC, 256], bf16, tag="wqk")
            nc.sync.dma_start(wqk_sb[:], wqk.rearrange("(ko p) m -> p ko m", p=128))
            wv_sb = cpool.tile([128, KC, 128], bf16, tag="wv")
            nc.sync.dma_start(wv_sb[:], wv.rearrange("(ko p) m -> p ko m", p=128))
            wo_sb = cpool.tile([128, KC, D], bf16, tag="wo")
            nc.sync.dma_start(wo_sb[:], wo.rearrange("(ko p) m -> p ko m", p=128))
            bqk_sb = cpool.tile([128, 2], f32, tag="bqk")
            nc.sync.dma_start(bqk_sb[:], bqk[:])
            bv_sb = cpool.tile([128, 128], bf16, tag="bv")
            nc.sync.dma_start(bv_sb[:], bv[:])
            bo_sb = cpool.tile([128, D], f32, tag="bo")
            nc.sync.dma_start(bo_sb[:], bo[:])
            mask_sb = cpool.tile([128, 896], bf16, tag="mask")
            nc.sync.dma_start(mask_sb[:], mask[:])

            # ---- persistent activations ----
            qT_sb = bigpool.tile([128, BS], bf16, tag="qT")   # [2*64 feat, B*S]
            kT_sb = bigpool.tile([128, BS], bf16, tag="kT")
            # v natural layout + ones cols: per 128-row chunk:
            #   [v_h0(0:64) | ones(64) | v_h1(65:129) | ones(129)]
            v_sb = bigpool.tile([128, BS // 128, 130], bf16, tag="v")
            ctxT_sb = bigpool.tile([128, BS], bf16, tag="ctxT")

            nc.vector.memset(v_sb[:, :, 64:65], 1.0)
            nc.vector.memset(v_sb[:, :, 129:130], 1.0)

            # ---- phase 1: qkv projections ----
            xT_r = xT.rearrange("(ko p) s -> p ko s", p=128)
            for sc in range(SC):
                xt = xpool.tile([128, KC, 512], bf16, tag="xt")
                nc.sync.dma_start(xt[:], xT_r[:, :, sc * 512:(sc + 1) * 512])

                ps_q = psA.tile([128, 1536], f32, tag="psA", name="ps_q")[:, :512]
                ps_k = psA.tile([128, 1536], f32, tag="psA", name="ps_k")[:, :512]
                for kk in range(KC):
                    nc.tensor.matmul(ps_q, lhsT=wqk_sb[:, kk, 0:128],
                                     rhs=xt[:, kk, :],
                                     start=(kk == 0), stop=(kk == KC - 1))
                for kk in range(KC):
                    nc.tensor.matmul(ps_k, lhsT=wqk_sb[:, kk, 128:256],
                                     rhs=xt[:, kk, :],
                                     start=(kk == 0), stop=(kk == KC - 1))
                qs = slice(sc * 512, (sc + 1) * 512)
                nc.vector.tensor_scalar_add(qT_sb[:, qs], ps_q, bqk_sb[:, 0:1])
                nc.vector.tensor_scalar_add(kT_sb[:, qs], ps_k, bqk_sb[:, 1:2])

                for s4 in range(4):
                    sidx = sc * 4 + s4
                    ps_v = psB.tile([128, 512], f32, tag="psB", name="ps_v")[:, :128]
                    for kk in range(KC):
                        nc.tensor.matmul(
                            ps_v,
                            lhsT=xt[:, kk, s4 * 128:(s4 + 1) * 128],
                            rhs=wv_sb[:, kk, :],
                            start=(kk == 0), stop=(kk == KC - 1))
                    nc.vector.tensor_tensor(v_sb[:, sidx, 0:64], ps_v[:, 0:64],
                                            bv_sb[:, 0:64], ALU.add)
                    nc.vector.tensor_tensor(v_sb[:, sidx, 65:129], ps_v[:, 64:128],
                                            bv_sb[:, 64:128], ALU.add)

            # ---- phase 2: attention per (batch, local head) ----
            for b in range(B):
                for hl in range(HPC):
                    hp = slice(64 * hl, 64 * hl + 64)   # feature partitions
                    exp_tiles = []
                    for t in range(NKT):
                        jmin = t // 4
                        width = S - 512 * jmin
                        et = epool.tile([128, width], bf16, tag=f"exp{t}")
                        kT_l = kT_sb[hp, b * S + t * 128: b * S + (t + 1) * 128]
                        off = 0
                        while off < width:
                            pw = min(1536, width - off)
                            ps = psA.tile([128, 1536], f32, tag="psA",
                                          name="ps_sc")[:, :pw]
                            for qi in range(pw // 512):
                                qlo = b * S + 512 * jmin + off + qi * 512
                                nc.tensor.matmul(
                                    ps[:, qi * 512:(qi + 1) * 512],
                                    lhsT=kT_l,
                                    rhs=qT_sb[hp, qlo:qlo + 512],
                                    start=True, stop=True)
                            nc.scalar.activation(et[:, off:off + pw], ps, ACTF.Exp)
                            off += pw
                        # causal mask on the diagonal 512-block
                        mo = 384 - 128 * (t % 4)
                        nc.vector.tensor_tensor(et[:, 0:512], et[:, 0:512],
                                                mask_sb[:, mo:mo + 512], ALU.mult)
                        exp_tiles.append(et)

                    # pass B: ctx^T = [v|1]^T @ exp, then normalize (detached)
                    for j in range(NQT):
                        ps_c = psB.tile([128, 512], f32, tag="psB", name="ps_c")
                        nkt = 4 * (j + 1)
                        for t in range(nkt):
                            qoff = 512 * (j - t // 4)
                            nc.tensor.matmul(
                                ps_c[:65, :],
                                lhsT=v_sb[:, b * NKT + t, 65 * hl: 65 * hl + 65],
                                rhs=exp_tiles[t][:, qoff:qoff + 512],
                                start=(t == 0), stop=(t == nkt - 1))
                        cs = slice(b * S + j * 512, b * S + (j + 1) * 512)
                        # stage out of PSUM fast, then normalize off the PE path
                        nc.vector.tensor_copy(ctxT_sb[hp, cs], ps_c[0:64, :])
                        den = spool.tile([1, 512], f32, tag="den")
                        nc.vector.tensor_copy(den[:], ps_c[64:65, :])
                        recip = spool.tile([1, 512], f32, tag="recip")
                        nc.vector.reciprocal_approx_fast(out=recip[:], in_=den[:])
                        bcast = spool.tile([128, 512], f32, tag="bcast")
                        nc.gpsimd.partition_broadcast(bcast[:], recip[:])
                        nc.vector.tensor_tensor(ctxT_sb[hp, cs], ctxT_sb[hp, cs],
                                                bcast[hp, :], ALU.mult)

                # batch b ctx^T complete on this core -> all-gather it
                nc.sync.dma_start(ctx_dram[b][:], ctxT_sb[:, b * S:(b + 1) * S])
                nc.gpsimd.collective_compute(
                    "AllGather",
                    mybir.AluOpType.bypass,
                    replica_groups=[list(range(NC))],
                    ins=[ctx_dram[b][:]],
                    outs=[ctxag_dram[b][:]],
                )

            # ---- phase 3: output projection ----
            # core c handles rows [256c, 256c+256) of each batch
            part = nc.partition_id()
            for b in range(B):
                ctxag_sb = bigpool.tile([128, NC, RPB], bf16, tag=f"ctxag{b}",
                                        name="ctxag_sb")
                ctxag_r = ctxag_dram[b].rearrange("(k p) s -> p k s", p=128)
                nc.sync.dma_start(
                    ctxag_sb[:], ctxag_r[:, :, bass.ds(part * RPB, RPB)])

                for rc in range(RPB // 128):
                    ot = opool.tile([128, D], f32, tag="ot")
                    for ncol in range(D // 512):
                        ps_o = psB.tile([128, 512], f32, tag="psB", name="ps_o")
                        for k in range(NC):
                            nc.tensor.matmul(
                                ps_o,
                                lhsT=ctxag_sb[:, k, rc * 128:(rc + 1) * 128],
                                rhs=wo_sb[:, k, ncol * 512:(ncol + 1) * 512],
                                start=(k == 0), stop=(k == NC - 1))
                        nc.vector.tensor_tensor(
                            ot[:, ncol * 512:(ncol + 1) * 512], ps_o,
                            bo_sb[:, ncol * 512:(ncol + 1) * 512], ALU.add)
                    nc.sync.dma_start(
                        out[b * RPB + rc * 128: b * RPB + (rc + 1) * 128, :],
                        ot[:])

    nc.compile()
    return nc


def _prep_inputs(x, Wqkv, bqkv, Wo, bo):
    x = np.asarray(x, dtype=np.float32)
    Wqkv = np.asarray(Wqkv, dtype=np.float32)
    bqkv = np.asarray(bqkv, dtype=np.float32)
    Wo = np.asarray(Wo, dtype=np.float32)
    bo = np.asarray(bo, dtype=np.float32)

    xT = np.ascontiguousarray(x.reshape(BS, D).T).astype(BF16)
    wo_b = Wo.astype(BF16)
    bo_t = np.tile(bo.astype(np.float32), (128, 1))

    kp = np.arange(128)[:, None]
    u = np.arange(896)[None, :]
    mask = (u >= 384 + kp).astype(BF16)

    scale = np.float32(1.0 / np.sqrt(HD))

    # Wqkv columns per head h: q = 192h..+64, k = +64, v = +128
    W3 = Wqkv.reshape(D, H, 3, HD)
    b3 = bqkv.reshape(H, 3, HD)

    in_maps = []
    for c in range(NC):
        hs = [HPC * c + i for i in range(HPC)]
        wq = np.concatenate([W3[:, h, 0, :] for h in hs], axis=1) * scale
        wk = np.concatenate([W3[:, h, 1, :] for h in hs], axis=1)
        wv_ = np.concatenate([W3[:, h, 2, :] for h in hs], axis=1)
        bq = np.concatenate([b3[h, 0, :] for h in hs]) * scale
        bk = np.concatenate([b3[h, 1, :] for h in hs])
        bv_ = np.concatenate([b3[h, 2, :] for h in hs])
        in_maps.append({
            "xT": xT,
            "wqk": np.ascontiguousarray(
                np.concatenate([wq, wk], axis=1)).astype(BF16),
            "wv": np.ascontiguousarray(wv_).astype(BF16),
            "wo": wo_b,
            "bqk": np.ascontiguousarray(
                np.stack([bq, bk], axis=1)).astype(np.float32),
            "bv": np.tile(bv_.astype(BF16), (128, 1)),
            "bo": bo_t,
            "mask": mask,
        })
    return in_maps


def run(x, Wqkv, bqkv, Wo, bo, trace=False):
    from concourse.bass_utils import run_bass_kernel_spmd

    if "nc" not in _CACHE:
        _CACHE["nc"] = _build_program()
    nc = _CACHE["nc"]
    in_maps = _prep_inputs(x, Wqkv, bqkv, Wo, bo)
    res = run_bass_kernel_spmd(nc, in_maps, list(range(NC)), trace=trace)
    # core c returns [2*RPB, D]: rows [256c,256c+256) of batch 0 then batch 1
    full = np.empty((B, S, D), dtype=np.float32)
    for c in range(NC):
        r = res.results[c]["out"]
        for b in range(B):
            full[b, RPB * c: RPB * (c + 1), :] = r[b * RPB:(b + 1) * RPB, :]
    return full, res


def kernel(x, Wqkv, bqkv, Wo, bo):
    out, _ = run(x, Wqkv, bqkv, Wo, bo)
    return out
